# revision 10
# baseline (speedup 1.0000x reference)
"""Bilateral filter (5x5, reflect pad) on 8 Trainium2 NeuronCores.

Contract: kernel(**inputs) takes the FULL inputs
  x:              [4, 3, 512, 512] f32
  spatial_kernel: [5, 5] f32
  sigma_color:    scalar f32
and returns the FULL output [4, 3, 512, 512] f32.

Sharding: pure data-parallel. The 12 images (B*C) are split into 24
half-images of 256 rows; each of the 8 cores gets 3 half-images with a
2-row halo (reflect padding applied on the host), i.e. input pieces of
[260, 516] producing output [256, 512].

Per-core kernel (per 128-row tile): load 5 vertically-shifted slabs of
the padded piece; the 25 taps are (vertical slab, horizontal AP offset)
pairs. For each tap:
    d  = p_t - c                      (DVE)
    q  = Square(gamma * d)            (ACT)   q = d^2 / (2 sigma_c^2)
    w  = Exp(-q + ln sk_t)            (ACT)   spatial weight folded into bias
    wp = w * p_t                      (DVE)
    S += w ; T += wp                  (DVE)
center tap is exact: w == 1, so S starts at 1 and T starts at c.
Output = T / (S + 1e-8).
"""

import os

import numpy as np

import bass_rust
import concourse.bacc as bacc
import concourse.bass as bass
import concourse.mybir as mybir
import concourse.tile as tile
from concourse import bass_utils

F32 = mybir.dt.float32
AF = mybir.ActivationFunctionType
ALU = mybir.AluOpType

N_CORES = 8
K = 5
PAD = 2
B, C, H, W = 4, 3, 512, 512
N_IMGS = B * C                    # 12
HALF_ROWS = 256                   # output rows per piece
PIECE_ROWS = HALF_ROWS + 2 * PAD  # 260
PIECE_COLS = W + 2 * PAD          # 516
PIECES_PER_CORE = (N_IMGS * 2) // N_CORES  # 3

_cached = {}


def _build(ln_sk: np.ndarray, gamma: float) -> bass.Bass:
    """Build the per-core Bass module (SPMD: same NEFF on all 8 cores)."""
    nc = bacc.Bacc("TRN2", target_bir_lowering=False, debug=False)
    x_in = nc.dram_tensor(
        "x_in", [PIECES_PER_CORE, PIECE_ROWS, PIECE_COLS], F32, kind="ExternalInput"
    ).ap()
    y_out = nc.dram_tensor(
        "y_out", [PIECES_PER_CORE, HALF_ROWS, W], F32, kind="ExternalOutput"
    ).ap()

    with tile.TileContext(nc) as tc:
        with (
            tc.tile_pool(name="const_pool", bufs=1) as const_pool,
            tc.tile_pool(name="slab_pool", bufs=2) as slab_pool,
            tc.tile_pool(name="work_pool", bufs=3) as work_pool,
            tc.tile_pool(name="acc_pool", bufs=2) as acc_pool,
        ):
            # per-tap ln(spatial_kernel) biases, one column per tap
            bias_tile = const_pool.tile([128, K * K], F32, tag="bias",
                                        name="lnsk_bias")
            for di in range(K):
                for dj in range(K):
                    tidx = di * K + dj
                    nc.gpsimd.memset(bias_tile[:, tidx : tidx + 1],
                                     float(ln_sk[di, dj]))
            for p in range(PIECES_PER_CORE):
                for t in range(2):  # two 128-row tiles per 256-row piece
                    r0 = t * 128
                    # One DMA loads all 5 vertically-shifted slabs as an
                    # overlapping-window read: dest [128, 5, 516], src row
                    # (r0 + part + di).  Single queue -> single wait sem.
                    slab = slab_pool.tile([128, K, PIECE_COLS], F32, tag="slab",
                                          name=f"slab_p{p}t{t}")
                    src_win = x_in[p, r0 : r0 + 128 + K - 1, :].copy()
                    src_win.ap = bass_rust.VecI64Pair(
                        [(PIECE_COLS, 128), (PIECE_COLS, K), (1, PIECE_COLS)]
                    )
                    nc.sync.dma_start(slab[:, :, :], src_win)
                    slabs = [slab[:, di, :] for di in range(K)]
                    c = slabs[PAD][:, PAD : PAD + W]

                    S = acc_pool.tile([128, W], F32, tag="S", name=f"S_p{p}t{t}")
                    T = acc_pool.tile([128, W], F32, tag="T", name=f"T_p{p}t{t}")
                    # center tap: d == 0 exactly -> w == 1 exactly
                    nc.gpsimd.memset(S[:, :], 1.0)
                    nc.scalar.copy(T[:, :], c)

                    for di in range(K):
                        for dj in range(K):
                            if di == PAD and dj == PAD:
                                continue
                            pt = slabs[di][:, dj : dj + W]
                            d = work_pool.tile([128, W], F32, tag="d",
                                               name=f"d_p{p}t{t}_{di}{dj}")
                            nc.vector.tensor_sub(d[:, :], pt, c)
                            q = work_pool.tile([128, W], F32, tag="q",
                                               name=f"q_p{p}t{t}_{di}{dj}")
                            nc.scalar.activation(q[:, :], d[:, :], AF.Square,
                                                 scale=float(gamma))
                            w = work_pool.tile([128, W], F32, tag="w",
                                               name=f"w_p{p}t{t}_{di}{dj}")
                            tidx = di * K + dj
                            nc.scalar.activation(w[:, :], q[:, :], AF.Exp,
                                                 bias=bias_tile[:, tidx : tidx + 1],
                                                 scale=-1.0)
                            wp = work_pool.tile([128, W], F32, tag="wp",
                                                name=f"wp_p{p}t{t}_{di}{dj}")
                            nc.vector.tensor_mul(wp[:, :], w[:, :], pt)
                            nc.vector.tensor_add(S[:, :], S[:, :], w[:, :])
                            nc.vector.tensor_add(T[:, :], T[:, :], wp[:, :])

                    Sp = work_pool.tile([128, W], F32, tag="Sp", name=f"Sp_p{p}t{t}")
                    nc.vector.tensor_scalar_add(Sp[:, :], S[:, :], 1e-8)
                    R = work_pool.tile([128, W], F32, tag="R", name=f"R_p{p}t{t}")
                    nc.vector.reciprocal(R[:, :], Sp[:, :])
                    out = work_pool.tile([128, W], F32, tag="out", name=f"out_p{p}t{t}")
                    nc.vector.tensor_mul(out[:, :], T[:, :], R[:, :])
                    nc.sync.dma_start(y_out[p, r0 : r0 + 128, :], out[:, :])
    nc.compile()
    return nc


def _get_nc(ln_sk: np.ndarray, gamma: float) -> bass.Bass:
    key = (ln_sk.tobytes(), float(gamma))
    if _cached.get("key") != key:
        _cached["key"] = key
        _cached["nc"] = _build(ln_sk, gamma)
    return _cached["nc"]


def kernel(x, spatial_kernel, sigma_color):
    x = np.ascontiguousarray(np.asarray(x, dtype=np.float32))
    sk = np.asarray(spatial_kernel, dtype=np.float64)
    sigma = float(np.asarray(sigma_color))

    gamma = 1.0 / (np.sqrt(2.0) * sigma)
    ln_sk = np.log(sk)

    imgs = x.reshape(N_IMGS, H, W)
    xp = np.pad(imgs, ((0, 0), (PAD, PAD), (PAD, PAD)), mode="reflect")
    # 24 half-image pieces with halo: [24, 260, 516]
    pieces = np.stack(
        [xp[:, 0:PIECE_ROWS, :], xp[:, HALF_ROWS : HALF_ROWS + PIECE_ROWS, :]],
        axis=1,
    ).reshape(N_IMGS * 2, PIECE_ROWS, PIECE_COLS)

    nc = _get_nc(ln_sk, gamma)
    in_maps = [
        {"x_in": np.ascontiguousarray(pieces[PIECES_PER_CORE * k : PIECES_PER_CORE * (k + 1)])}
        for k in range(N_CORES)
    ]
    trace = os.environ.get("BILATERAL_TRACE", "0") == "1"
    res = bass_utils.run_bass_kernel_spmd(
        nc, in_maps, core_ids=list(range(N_CORES)), trace=trace
    )
    kernel.last_results = res

    outs = np.stack([res.results[k]["y_out"] for k in range(N_CORES)])
    out = outs.reshape(N_IMGS, 2, HALF_ROWS, W).reshape(N_IMGS, H, W)
    return out.reshape(B, C, H, W).astype(np.float32)


kernel.last_results = None


# revision 15
# speedup vs baseline: 1.8521x; 1.8521x over previous
"""Bilateral filter (5x5, reflect pad) on 8 Trainium2 NeuronCores.

Contract: kernel(**inputs) takes the FULL inputs
  x:              [4, 3, 512, 512] f32
  spatial_kernel: [5, 5] f32
  sigma_color:    scalar f32
and returns the FULL output [4, 3, 512, 512] f32.

Sharding: pure data-parallel. The 12 images (B*C) are split into 24
half-images of 256 rows; each of the 8 cores gets 3 half-images with a
2-row halo (reflect padding applied on the host): input pieces of
[260, 516] producing output [256, 512].

Per-core kernel, per 128-row tile:
  - one DMA loads 5 vertically-shifted slabs (overlapping-window AP)
  - per tap t=(di,dj), 24 non-center taps:
        d  = p_t - c                    (DVE TT sub)
        q  = Square(gamma * d)          (ACT)    q = d^2/(2 sigma_c^2)
        w  = Exp(-q + ln sk_t)          (ACT)    spatial weight in bias
        wp = w * p_t                    (DVE TT mul)
        S_psum += I @ w                 (PE identity matmul, PSUM acc)
        T_psum += I @ wp                (PE identity matmul, PSUM acc)
  - center tap is exact (w=1, wp=c) and is folded into the epilogue:
        S   = S_psum + (1 + 1e-8)
        T   = T_psum + c
        out = T * reciprocal(S)
The TensorEngine does both tap-reductions (accumulating identity
matmuls into PSUM), keeping the DVE to 2 streaming ops per tap.
"""

import os

import numpy as np

import bass_rust
import concourse.bacc as bacc
import concourse.bass as bass
import concourse.mybir as mybir
import concourse.tile as tile
from concourse import bass_utils

F32 = mybir.dt.float32
BF16 = mybir.dt.bfloat16
AF = mybir.ActivationFunctionType
ALU = mybir.AluOpType

N_CORES = 8
K = 5
PAD = 2
B, C, H, W = 4, 3, 512, 512
N_IMGS = B * C                    # 12
HALF_ROWS = 256                   # output rows per piece
PIECE_ROWS = HALF_ROWS + 2 * PAD  # 260
PIECE_COLS = W + 2 * PAD          # 516
PIECES_PER_CORE = (N_IMGS * 2) // N_CORES  # 3

TAPS = [(di, dj) for di in range(K) for dj in range(K)
        if not (di == PAD and dj == PAD)]

_cached = {}


def _build(ln_sk: np.ndarray, gamma: float) -> bass.Bass:
    """Build the per-core Bass module (SPMD: same NEFF on all 8 cores)."""
    nc = bacc.Bacc("TRN2", target_bir_lowering=False, debug=False)
    x_in = nc.dram_tensor(
        "x_in", [PIECES_PER_CORE, PIECE_ROWS, PIECE_COLS], F32, kind="ExternalInput"
    ).ap()
    ident_in = nc.dram_tensor("ident", [128, 128], F32, kind="ExternalInput").ap()
    y_out = nc.dram_tensor(
        "y_out", [PIECES_PER_CORE, HALF_ROWS, W], F32, kind="ExternalOutput"
    ).ap()

    with tile.TileContext(nc) as tc:
        with (
            tc.tile_pool(name="const_pool", bufs=1) as const_pool,
            tc.tile_pool(name="slab_pool", bufs=2) as slab_pool,
            tc.tile_pool(name="work_pool", bufs=3) as work_pool,
            tc.tile_pool(name="epi_pool", bufs=2) as epi_pool,
            tc.tile_pool(name="psum_pool", bufs=2, space="PSUM") as psum_pool,
        ):
            # per-tap ln(spatial_kernel) biases, one column per tap
            bias_tile = const_pool.tile([128, K * K], F32, tag="bias",
                                        name="lnsk_bias")
            for tidx, (di, dj) in enumerate(TAPS):
                nc.gpsimd.memset(bias_tile[:, tidx : tidx + 1],
                                 float(ln_sk[di, dj]))
            ident_f = const_pool.tile([128, 128], F32, tag="ident_f",
                                      name="ident_f")
            nc.sync.dma_start(ident_f[:, :], ident_in)
            ident = const_pool.tile([128, 128], BF16, tag="ident", name="ident")
            nc.vector.tensor_copy(ident[:, :], ident_f[:, :])

            for p in range(PIECES_PER_CORE):
                for t in range(2):  # two 128-row tiles per 256-row piece
                    r0 = t * 128
                    # One DMA loads all 5 vertically-shifted slabs as an
                    # overlapping-window read: dest [128, 5, 516], src row
                    # (r0 + part + di).
                    slab = slab_pool.tile([128, K, PIECE_COLS], F32, tag="slab",
                                          name=f"slab_p{p}t{t}")
                    src_win = x_in[p, r0 : r0 + 128 + K - 1, :].copy()
                    src_win.ap = bass_rust.VecI64Pair(
                        [(PIECE_COLS, 128), (PIECE_COLS, K), (1, PIECE_COLS)]
                    )
                    nc.sync.dma_start(slab[:, :, :], src_win)
                    c = slab[:, PAD, PAD : PAD + W]
                    # bf16 copy of the slab for the (bf16 x bf16) wp multiply
                    slab16 = slab_pool.tile([128, K, PIECE_COLS], BF16,
                                            tag="slab16", name=f"slab16_p{p}t{t}")
                    nc.vector.tensor_copy(slab16[:, :, :], slab[:, :, :])

                    S_ps = psum_pool.tile([128, W], F32, tag="S",
                                          name=f"Sps_p{p}t{t}")
                    T_ps = psum_pool.tile([128, W], F32, tag="T",
                                          name=f"Tps_p{p}t{t}")

                    n_taps = len(TAPS)
                    for tidx, (di, dj) in enumerate(TAPS):
                        pt = slab[:, di, dj : dj + W]
                        pt16 = slab16[:, di, dj : dj + W]
                        d = work_pool.tile([128, W], F32, tag="d",
                                           name=f"d_p{p}t{t}_{di}{dj}")
                        nc.vector.tensor_sub(d[:, :], pt, c)
                        q = work_pool.tile([128, W], F32, tag="q",
                                           name=f"q_p{p}t{t}_{di}{dj}")
                        nc.scalar.activation(q[:, :], d[:, :], AF.Square,
                                             scale=float(gamma))
                        w = work_pool.tile([128, W], BF16, tag="w", bufs=4,
                                           name=f"w_p{p}t{t}_{di}{dj}")
                        nc.scalar.activation(w[:, :], q[:, :], AF.Exp,
                                             bias=bias_tile[:, tidx : tidx + 1],
                                             scale=-1.0)
                        wp = work_pool.tile([128, W], BF16, tag="wp", bufs=4,
                                            name=f"wp_p{p}t{t}_{di}{dj}")
                        nc.vector.tensor_mul(wp[:, :], w[:, :], pt16)
                        nc.tensor.matmul(S_ps[:, :], ident[:, :], w[:, :],
                                         start=(tidx == 0),
                                         stop=(tidx == n_taps - 1))
                        nc.tensor.matmul(T_ps[:, :], ident[:, :], wp[:, :],
                                         start=(tidx == 0),
                                         stop=(tidx == n_taps - 1))

                    # epilogue: fold exact center tap (w=1, wp=c) + epsilon
                    S = epi_pool.tile([128, W], F32, tag="S", name=f"S_p{p}t{t}")
                    nc.vector.tensor_scalar(S[:, :], S_ps[:, :],
                                            float(1.0 + 1e-8), None, op0=ALU.add)
                    Tc = epi_pool.tile([128, W], F32, tag="Tc", name=f"Tc_p{p}t{t}")
                    nc.vector.tensor_add(Tc[:, :], T_ps[:, :], c)
                    R = epi_pool.tile([128, W], F32, tag="R", name=f"R_p{p}t{t}")
                    scr = epi_pool.tile([128, W], F32, tag="scr",
                                        name=f"scr_p{p}t{t}")
                    nc.vector.reciprocal_approx_accurate(R[:, :], S[:, :],
                                                         scr[:, :])
                    out = epi_pool.tile([128, W], F32, tag="out",
                                        name=f"out_p{p}t{t}")
                    nc.vector.tensor_mul(out[:, :], Tc[:, :], R[:, :])
                    nc.sync.dma_start(y_out[p, r0 : r0 + 128, :], out[:, :])
    nc.compile()
    return nc


def _get_nc(ln_sk: np.ndarray, gamma: float) -> bass.Bass:
    key = (ln_sk.tobytes(), float(gamma))
    if _cached.get("key") != key:
        _cached["key"] = key
        _cached["nc"] = _build(ln_sk, gamma)
    return _cached["nc"]


def kernel(x, spatial_kernel, sigma_color):
    x = np.ascontiguousarray(np.asarray(x, dtype=np.float32))
    sk = np.asarray(spatial_kernel, dtype=np.float64)
    sigma = float(np.asarray(sigma_color))

    gamma = 1.0 / (np.sqrt(2.0) * sigma)
    ln_sk = np.log(sk)

    imgs = x.reshape(N_IMGS, H, W)
    xp = np.pad(imgs, ((0, 0), (PAD, PAD), (PAD, PAD)), mode="reflect")
    # 24 half-image pieces with halo: [24, 260, 516]
    pieces = np.stack(
        [xp[:, 0:PIECE_ROWS, :], xp[:, HALF_ROWS : HALF_ROWS + PIECE_ROWS, :]],
        axis=1,
    ).reshape(N_IMGS * 2, PIECE_ROWS, PIECE_COLS)

    nc = _get_nc(ln_sk, gamma)
    ident = np.eye(128, dtype=np.float32)
    in_maps = [
        {
            "x_in": np.ascontiguousarray(
                pieces[PIECES_PER_CORE * k : PIECES_PER_CORE * (k + 1)]
            ),
            "ident": ident,
        }
        for k in range(N_CORES)
    ]
    trace = os.environ.get("BILATERAL_TRACE", "0") == "1"
    res = bass_utils.run_bass_kernel_spmd(
        nc, in_maps, core_ids=list(range(N_CORES)), trace=trace
    )
    kernel.last_results = res

    outs = np.stack([res.results[k]["y_out"] for k in range(N_CORES)])
    out = outs.reshape(N_IMGS, 2, HALF_ROWS, W).reshape(N_IMGS, H, W)
    return out.reshape(B, C, H, W).astype(np.float32)


kernel.last_results = None


# revision 16
# speedup vs baseline: 2.0221x; 1.0918x over previous
"""Bilateral filter (5x5, reflect pad) on 8 Trainium2 NeuronCores.

Contract: kernel(**inputs) takes the FULL inputs
  x:              [4, 3, 512, 512] f32
  spatial_kernel: [5, 5] f32
  sigma_color:    scalar f32
and returns the FULL output [4, 3, 512, 512] f32.

Sharding: pure data-parallel. The 12 images (B*C) are split into 24
half-images of 256 rows; each of the 8 cores gets 3 half-images with a
2-row halo (reflect padding applied on the host): input pieces of
[260, 516] producing output [256, 512].

Per-core kernel, per 128-row tile (24 non-center taps in 3 groups of 8):
  - one DMA loads 5 vertically-shifted slabs (overlapping-window AP)
  - per tap: d = p_t - c (DVE, f32)  into a group ring
  - per group of 8 taps (batched ACT, one instruction each):
        q = Square(gamma * d)    [128, 8*512]
        w = Exp(-q)        f32->bf16, no bias
  - per tap: wp = w * p16 (DVE bf16), then two PE matmuls with a
    sk_t-scaled identity as lhsT accumulate into PSUM:
        S_psum += sk_t * w ;  T_psum += sk_t * wp
  - center tap is exact (w=1, wp=c), folded into the epilogue:
        out = (T + c) * reciprocal(S + 1 + 1e-8)
The TensorEngine does both tap-reductions AND applies the spatial
weights (scaled-identity matmuls, f32 PSUM accumulation); batching the
activations 8-wide amortizes the ACT per-instruction overhead.
"""

import os

import numpy as np

import bass_rust
import concourse.bacc as bacc
import concourse.bass as bass
import concourse.mybir as mybir
import concourse.tile as tile
from concourse import bass_utils

F32 = mybir.dt.float32
BF16 = mybir.dt.bfloat16
AF = mybir.ActivationFunctionType
ALU = mybir.AluOpType

N_CORES = 8
K = 5
PAD = 2
B, C, H, W = 4, 3, 512, 512
N_IMGS = B * C                    # 12
HALF_ROWS = 256                   # output rows per piece
PIECE_ROWS = HALF_ROWS + 2 * PAD  # 260
PIECE_COLS = W + 2 * PAD          # 516
PIECES_PER_CORE = (N_IMGS * 2) // N_CORES  # 3

TAPS = [(di, dj) for di in range(K) for dj in range(K)
        if not (di == PAD and dj == PAD)]
GROUP = 8
N_GROUPS = len(TAPS) // GROUP     # 3

_cached = {}


def _build(sk: np.ndarray, gamma: float) -> bass.Bass:
    """Build the per-core Bass module (SPMD: same NEFF on all 8 cores)."""
    nc = bacc.Bacc("TRN2", target_bir_lowering=False, debug=False)
    x_in = nc.dram_tensor(
        "x_in", [PIECES_PER_CORE, PIECE_ROWS, PIECE_COLS], F32, kind="ExternalInput"
    ).ap()
    ident_in = nc.dram_tensor("ident", [128, 128], F32, kind="ExternalInput").ap()
    y_out = nc.dram_tensor(
        "y_out", [PIECES_PER_CORE, HALF_ROWS, W], F32, kind="ExternalOutput"
    ).ap()

    with tile.TileContext(nc) as tc:
        with (
            tc.tile_pool(name="const_pool", bufs=1) as const_pool,
            tc.tile_pool(name="slab_pool", bufs=2) as slab_pool,
            tc.tile_pool(name="work_pool", bufs=2) as work_pool,
            tc.tile_pool(name="epi_pool", bufs=2) as epi_pool,
            tc.tile_pool(name="psum_pool", bufs=2, space="PSUM") as psum_pool,
        ):
            # sk_t-scaled identity matrices (bf16) as matmul weights
            ident_f = const_pool.tile([128, 128], F32, tag="ident_f",
                                      name="ident_f")
            nc.sync.dma_start(ident_f[:, :], ident_in)
            skI = const_pool.tile([128, len(TAPS), 128], BF16, tag="skI",
                                  name="skI")
            skI_f = const_pool.tile([128, 128], F32, tag="skI_f", name="skI_f")
            for tidx, (di, dj) in enumerate(TAPS):
                nc.vector.tensor_scalar(skI_f[:, :], ident_f[:, :],
                                        float(sk[di, dj]), None, op0=ALU.mult)
                nc.vector.tensor_copy(skI[:, tidx, :], skI_f[:, :])

            for p in range(PIECES_PER_CORE):
                for t in range(2):  # two 128-row tiles per 256-row piece
                    r0 = t * 128
                    # One DMA loads all 5 vertically-shifted slabs as an
                    # overlapping-window read: dest [128, 5, 516], src row
                    # (r0 + part + di).
                    slab = slab_pool.tile([128, K, PIECE_COLS], F32, tag="slab",
                                          name=f"slab_p{p}t{t}")
                    src_win = x_in[p, r0 : r0 + 128 + K - 1, :].copy()
                    src_win.ap = bass_rust.VecI64Pair(
                        [(PIECE_COLS, 128), (PIECE_COLS, K), (1, PIECE_COLS)]
                    )
                    nc.sync.dma_start(slab[:, :, :], src_win)
                    c = slab[:, PAD, PAD : PAD + W]
                    # bf16 copy of the slab for the (bf16 x bf16) wp multiply
                    slab16 = slab_pool.tile([128, K, PIECE_COLS], BF16,
                                            tag="slab16", name=f"slab16_p{p}t{t}")
                    nc.vector.tensor_copy(slab16[:, :, :], slab[:, :, :])

                    S_ps = psum_pool.tile([128, W], F32, tag="S",
                                          name=f"Sps_p{p}t{t}")
                    T_ps = psum_pool.tile([128, W], F32, tag="T",
                                          name=f"Tps_p{p}t{t}")

                    for g in range(N_GROUPS):
                        gtaps = TAPS[g * GROUP : (g + 1) * GROUP]
                        d_ring = work_pool.tile([128, GROUP, W], F32, tag="d",
                                                name=f"d_p{p}t{t}g{g}")
                        for j, (di, dj) in enumerate(gtaps):
                            nc.vector.tensor_sub(d_ring[:, j, :],
                                                 slab[:, di, dj : dj + W], c)
                        q_ring = work_pool.tile([128, GROUP, W], F32, tag="q",
                                                name=f"q_p{p}t{t}g{g}")
                        nc.scalar.activation(q_ring[:, :, :], d_ring[:, :, :],
                                             AF.Square, scale=float(gamma))
                        w_ring = work_pool.tile([128, GROUP, W], BF16, tag="w",
                                                name=f"w_p{p}t{t}g{g}")
                        nc.scalar.activation(w_ring[:, :, :], q_ring[:, :, :],
                                             AF.Exp, scale=-1.0)
                        for j, (di, dj) in enumerate(gtaps):
                            tidx = g * GROUP + j
                            first = tidx == 0
                            last = tidx == len(TAPS) - 1
                            wp = work_pool.tile([128, W], BF16, tag="wp", bufs=4,
                                                name=f"wp_p{p}t{t}_{di}{dj}")
                            nc.vector.tensor_mul(wp[:, :], w_ring[:, j, :],
                                                 slab16[:, di, dj : dj + W])
                            nc.tensor.matmul(S_ps[:, :], skI[:, tidx, :],
                                             w_ring[:, j, :],
                                             start=first, stop=last)
                            nc.tensor.matmul(T_ps[:, :], skI[:, tidx, :],
                                             wp[:, :],
                                             start=first, stop=last)

                    # epilogue: fold exact center tap (w=1, wp=c) + epsilon
                    S = epi_pool.tile([128, W], F32, tag="S", name=f"S_p{p}t{t}")
                    nc.vector.tensor_scalar(S[:, :], S_ps[:, :],
                                            float(1.0 + 1e-8), None, op0=ALU.add)
                    Tc = epi_pool.tile([128, W], F32, tag="Tc", name=f"Tc_p{p}t{t}")
                    nc.vector.tensor_add(Tc[:, :], T_ps[:, :], c)
                    R = epi_pool.tile([128, W], F32, tag="R", name=f"R_p{p}t{t}")
                    scr = epi_pool.tile([128, W], F32, tag="scr",
                                        name=f"scr_p{p}t{t}")
                    nc.vector.reciprocal_approx_accurate(R[:, :], S[:, :],
                                                         scr[:, :])
                    out = epi_pool.tile([128, W], F32, tag="out",
                                        name=f"out_p{p}t{t}")
                    nc.vector.tensor_mul(out[:, :], Tc[:, :], R[:, :])
                    nc.sync.dma_start(y_out[p, r0 : r0 + 128, :], out[:, :])
    nc.compile()
    return nc


def _get_nc(sk: np.ndarray, gamma: float) -> bass.Bass:
    key = (sk.tobytes(), float(gamma))
    if _cached.get("key") != key:
        _cached["key"] = key
        _cached["nc"] = _build(sk, gamma)
    return _cached["nc"]


def kernel(x, spatial_kernel, sigma_color):
    x = np.ascontiguousarray(np.asarray(x, dtype=np.float32))
    sk = np.asarray(spatial_kernel, dtype=np.float64)
    sigma = float(np.asarray(sigma_color))

    gamma = 1.0 / (np.sqrt(2.0) * sigma)

    imgs = x.reshape(N_IMGS, H, W)
    xp = np.pad(imgs, ((0, 0), (PAD, PAD), (PAD, PAD)), mode="reflect")
    # 24 half-image pieces with halo: [24, 260, 516]
    pieces = np.stack(
        [xp[:, 0:PIECE_ROWS, :], xp[:, HALF_ROWS : HALF_ROWS + PIECE_ROWS, :]],
        axis=1,
    ).reshape(N_IMGS * 2, PIECE_ROWS, PIECE_COLS)

    nc = _get_nc(sk, gamma)
    ident = np.eye(128, dtype=np.float32)
    in_maps = [
        {
            "x_in": np.ascontiguousarray(
                pieces[PIECES_PER_CORE * k : PIECES_PER_CORE * (k + 1)]
            ),
            "ident": ident,
        }
        for k in range(N_CORES)
    ]
    trace = os.environ.get("BILATERAL_TRACE", "0") == "1"
    res = bass_utils.run_bass_kernel_spmd(
        nc, in_maps, core_ids=list(range(N_CORES)), trace=trace
    )
    kernel.last_results = res

    outs = np.stack([res.results[k]["y_out"] for k in range(N_CORES)])
    out = outs.reshape(N_IMGS, 2, HALF_ROWS, W).reshape(N_IMGS, H, W)
    return out.reshape(B, C, H, W).astype(np.float32)


kernel.last_results = None


# revision 18
# speedup vs baseline: 2.1872x; 1.0816x over previous
"""Bilateral filter (5x5, reflect pad) on 8 Trainium2 NeuronCores.

Contract: kernel(**inputs) takes the FULL inputs
  x:              [4, 3, 512, 512] f32
  spatial_kernel: [5, 5] f32
  sigma_color:    scalar f32
and returns the FULL output [4, 3, 512, 512] f32.

Sharding: pure data-parallel. The 12 images (B*C) are split into 24
half-images of 256 rows; each of the 8 cores gets 3 half-images with a
2-row halo (reflect padding applied on the host): input pieces of
[260, 516] producing output [256, 512].

Per-core kernel, per 128-row tile (24 non-center taps in 3 groups of 8):
  - one DMA loads 5 vertically-shifted slabs (overlapping-window AP)
  - per tap: d = p_t - c (DVE, f32)  into a group ring
  - per group of 8 taps (batched ACT, one instruction each):
        q = Square(gamma * d)    [128, 8*512]
        w = Exp(-q)        f32->bf16, no bias
  - per tap: wp = w * p16 (DVE bf16), then two PE matmuls with a
    sk_t-scaled identity as lhsT accumulate into PSUM:
        S_psum += sk_t * w ;  T_psum += sk_t * wp
  - center tap is exact (w=1, wp=c), folded into the epilogue:
        out = (T + c) * reciprocal(S + 1 + 1e-8)
The TensorEngine does both tap-reductions AND applies the spatial
weights (scaled-identity matmuls, f32 PSUM accumulation); batching the
activations 8-wide amortizes the ACT per-instruction overhead.
"""

import os

import numpy as np

import bass_rust
import concourse.bacc as bacc
import concourse.bass as bass
import concourse.mybir as mybir
import concourse.tile as tile
from concourse import bass_utils

F32 = mybir.dt.float32
BF16 = mybir.dt.bfloat16
AF = mybir.ActivationFunctionType
ALU = mybir.AluOpType

N_CORES = 8
K = 5
PAD = 2
B, C, H, W = 4, 3, 512, 512
N_IMGS = B * C                    # 12
HALF_ROWS = 256                   # output rows per piece
PIECE_ROWS = HALF_ROWS + 2 * PAD  # 260
PIECE_COLS = W + 2 * PAD          # 516
PIECES_PER_CORE = (N_IMGS * 2) // N_CORES  # 3

# taps grouped by uniform-stride runs of dj (whole rows; the center row
# splits around the center tap) so subs/muls batch into single DVE ops
# with overlapping-window APs
ROW_GROUPS = [
    (0, [0, 1, 2, 3, 4]),
    (1, [0, 1, 2, 3, 4]),
    (3, [0, 1, 2, 3, 4]),
    (4, [0, 1, 2, 3, 4]),
    (2, [0, 1]),
    (2, [3, 4]),
]
TAPS = [(di, dj) for di, djs in ROW_GROUPS for dj in djs]

_cached = {}


def _build(sk: np.ndarray, gamma: float) -> bass.Bass:
    """Build the per-core Bass module (SPMD: same NEFF on all 8 cores)."""
    nc = bacc.Bacc("TRN2", target_bir_lowering=False, debug=False)
    x_in = nc.dram_tensor(
        "x_in", [PIECES_PER_CORE, PIECE_ROWS, PIECE_COLS], F32, kind="ExternalInput"
    ).ap()
    ident_in = nc.dram_tensor("ident", [128, 128], F32, kind="ExternalInput").ap()
    y_out = nc.dram_tensor(
        "y_out", [PIECES_PER_CORE, HALF_ROWS, W], F32, kind="ExternalOutput"
    ).ap()

    with tile.TileContext(nc) as tc:
        with (
            tc.tile_pool(name="const_pool", bufs=1) as const_pool,
            tc.tile_pool(name="slab_pool", bufs=2) as slab_pool,
            tc.tile_pool(name="work_pool", bufs=2) as work_pool,
            tc.tile_pool(name="epi_pool", bufs=2) as epi_pool,
            tc.tile_pool(name="psum_pool", bufs=2, space="PSUM") as psum_pool,
        ):
            # sk_t-scaled identity matrices (bf16) as matmul weights
            ident_f = const_pool.tile([128, 128], F32, tag="ident_f",
                                      name="ident_f")
            nc.sync.dma_start(ident_f[:, :], ident_in)
            skI = const_pool.tile([128, len(TAPS), 128], BF16, tag="skI",
                                  name="skI")
            skI_f = const_pool.tile([128, 128], F32, tag="skI_f", name="skI_f")
            for tidx, (di, dj) in enumerate(TAPS):
                nc.vector.tensor_scalar(skI_f[:, :], ident_f[:, :],
                                        float(sk[di, dj]), None, op0=ALU.mult)
                nc.vector.tensor_copy(skI[:, tidx, :], skI_f[:, :])

            for p in range(PIECES_PER_CORE):
                for t in range(2):  # two 128-row tiles per 256-row piece
                    r0 = t * 128
                    # One DMA loads all 5 vertically-shifted slabs as an
                    # overlapping-window read: dest [128, 5, 516], src row
                    # (r0 + part + di).
                    slab = slab_pool.tile([128, K, PIECE_COLS], F32, tag="slab",
                                          name=f"slab_p{p}t{t}")
                    src_win = x_in[p, r0 : r0 + 128 + K - 1, :].copy()
                    src_win.ap = bass_rust.VecI64Pair(
                        [(PIECE_COLS, 128), (PIECE_COLS, K), (1, PIECE_COLS)]
                    )
                    nc.sync.dma_start(slab[:, :, :], src_win)
                    c = slab[:, PAD, PAD : PAD + W]
                    # bf16 copy of the slab for the (bf16 x bf16) wp multiply
                    # (on ACT: the DVE is the busier engine)
                    slab16 = slab_pool.tile([128, K, PIECE_COLS], BF16,
                                            tag="slab16", name=f"slab16_p{p}t{t}")
                    nc.scalar.copy(slab16[:, :, :], slab[:, :, :])

                    S_ps = psum_pool.tile([128, W], F32, tag="S",
                                          name=f"Sps_p{p}t{t}")
                    T_ps = psum_pool.tile([128, W], F32, tag="T",
                                          name=f"Tps_p{p}t{t}")

                    def win(base_tile, ncols, di, dj0, g):
                        """overlapping-window AP [128, g, 512]: tap dim strides
                        1 column."""
                        v = base_tile[:, 0, 0:W].copy()
                        v.ap = bass_rust.VecI64Pair(
                            [(K * ncols, 128), (1, g), (1, W)]
                        )
                        v.offset = base_tile[:, :, :].offset + di * ncols + dj0
                        return v

                    tidx = 0
                    for di, djs in ROW_GROUPS:
                        g = len(djs)
                        dj0 = djs[0]
                        d_ring = work_pool.tile([128, g, W], F32, tag="d",
                                                padded_shape=[128, K, W],
                                                name=f"d_p{p}t{t}r{di}_{dj0}")
                        src = win(slab, PIECE_COLS, di, dj0, g)
                        cb = c.copy()
                        cb.ap = bass_rust.VecI64Pair(
                            [(K * PIECE_COLS, 128), (0, g), (1, W)]
                        )
                        nc.vector.tensor_sub(d_ring[:, :, :], src, cb)
                        q_ring = work_pool.tile([128, g, W], F32, tag="q",
                                                padded_shape=[128, K, W],
                                                name=f"q_p{p}t{t}r{di}_{dj0}")
                        nc.scalar.activation(q_ring[:, :, :], d_ring[:, :, :],
                                             AF.Square, scale=float(gamma))
                        w_ring = work_pool.tile([128, g, W], BF16, tag="w",
                                                padded_shape=[128, K, W],
                                                name=f"w_p{p}t{t}r{di}_{dj0}")
                        nc.scalar.activation(w_ring[:, :, :], q_ring[:, :, :],
                                             AF.Exp, scale=-1.0)
                        wp_ring = work_pool.tile([128, g, W], BF16, tag="wp",
                                                 padded_shape=[128, K, W],
                                                 name=f"wp_p{p}t{t}r{di}_{dj0}")
                        nc.vector.tensor_mul(wp_ring[:, :, :], w_ring[:, :, :],
                                             win(slab16, PIECE_COLS, di, dj0, g))
                        for j in range(g):
                            first = tidx == 0
                            last = tidx == len(TAPS) - 1
                            nc.tensor.matmul(S_ps[:, :], skI[:, tidx, :],
                                             w_ring[:, j, :],
                                             start=first, stop=last)
                            nc.tensor.matmul(T_ps[:, :], skI[:, tidx, :],
                                             wp_ring[:, j, :],
                                             start=first, stop=last)
                            tidx += 1

                    # epilogue: fold exact center tap (w=1, wp=c) + epsilon
                    S = epi_pool.tile([128, W], F32, tag="S", name=f"S_p{p}t{t}")
                    nc.vector.tensor_scalar(S[:, :], S_ps[:, :],
                                            float(1.0 + 1e-8), None, op0=ALU.add)
                    Tc = epi_pool.tile([128, W], F32, tag="Tc", name=f"Tc_p{p}t{t}")
                    nc.vector.tensor_add(Tc[:, :], T_ps[:, :], c)
                    R = epi_pool.tile([128, W], F32, tag="R", name=f"R_p{p}t{t}")
                    scr = epi_pool.tile([128, W], F32, tag="scr",
                                        name=f"scr_p{p}t{t}")
                    nc.vector.reciprocal_approx_accurate(R[:, :], S[:, :],
                                                         scr[:, :])
                    out = epi_pool.tile([128, W], F32, tag="out",
                                        name=f"out_p{p}t{t}")
                    nc.vector.tensor_mul(out[:, :], Tc[:, :], R[:, :])
                    nc.sync.dma_start(y_out[p, r0 : r0 + 128, :], out[:, :])
    nc.compile()
    return nc


def _get_nc(sk: np.ndarray, gamma: float) -> bass.Bass:
    key = (sk.tobytes(), float(gamma))
    if _cached.get("key") != key:
        _cached["key"] = key
        _cached["nc"] = _build(sk, gamma)
    return _cached["nc"]


def kernel(x, spatial_kernel, sigma_color):
    x = np.ascontiguousarray(np.asarray(x, dtype=np.float32))
    sk = np.asarray(spatial_kernel, dtype=np.float64)
    sigma = float(np.asarray(sigma_color))

    gamma = 1.0 / (np.sqrt(2.0) * sigma)

    imgs = x.reshape(N_IMGS, H, W)
    xp = np.pad(imgs, ((0, 0), (PAD, PAD), (PAD, PAD)), mode="reflect")
    # 24 half-image pieces with halo: [24, 260, 516]
    pieces = np.stack(
        [xp[:, 0:PIECE_ROWS, :], xp[:, HALF_ROWS : HALF_ROWS + PIECE_ROWS, :]],
        axis=1,
    ).reshape(N_IMGS * 2, PIECE_ROWS, PIECE_COLS)

    nc = _get_nc(sk, gamma)
    ident = np.eye(128, dtype=np.float32)
    in_maps = [
        {
            "x_in": np.ascontiguousarray(
                pieces[PIECES_PER_CORE * k : PIECES_PER_CORE * (k + 1)]
            ),
            "ident": ident,
        }
        for k in range(N_CORES)
    ]
    trace = os.environ.get("BILATERAL_TRACE", "0") == "1"
    res = bass_utils.run_bass_kernel_spmd(
        nc, in_maps, core_ids=list(range(N_CORES)), trace=trace
    )
    kernel.last_results = res

    outs = np.stack([res.results[k]["y_out"] for k in range(N_CORES)])
    out = outs.reshape(N_IMGS, 2, HALF_ROWS, W).reshape(N_IMGS, H, W)
    return out.reshape(B, C, H, W).astype(np.float32)


kernel.last_results = None


# revision 20
# speedup vs baseline: 2.3072x; 1.0548x over previous
"""Bilateral filter (5x5, reflect pad) on 8 Trainium2 NeuronCores.

Contract: kernel(**inputs) takes the FULL inputs
  x:              [4, 3, 512, 512] f32
  spatial_kernel: [5, 5] f32
  sigma_color:    scalar f32
and returns the FULL output [4, 3, 512, 512] f32.

Sharding: pure data-parallel. The 12 images (B*C) are split into 24
half-images of 256 rows; each of the 8 cores gets 3 half-images with a
2-row halo (reflect padding applied on the host): input pieces of
[260, 516] producing output [256, 512].

Per-core kernel, per 128-row tile (24 non-center taps in 3 groups of 8):
  - one DMA loads 5 vertically-shifted slabs (overlapping-window AP)
  - per tap: d = p_t - c (DVE, f32)  into a group ring
  - per group of 8 taps (batched ACT, one instruction each):
        q = Square(gamma * d)    [128, 8*512]
        w = Exp(-q)        f32->bf16, no bias
  - per tap: wp = w * p16 (DVE bf16), then two PE matmuls with a
    sk_t-scaled identity as lhsT accumulate into PSUM:
        S_psum += sk_t * w ;  T_psum += sk_t * wp
  - center tap is exact (w=1, wp=c), folded into the epilogue:
        out = (T + c) * reciprocal(S + 1 + 1e-8)
The TensorEngine does both tap-reductions AND applies the spatial
weights (scaled-identity matmuls, f32 PSUM accumulation); batching the
activations 8-wide amortizes the ACT per-instruction overhead.
"""

import os

import numpy as np

import bass_rust
import concourse.bacc as bacc
import concourse.bass as bass
import concourse.mybir as mybir
import concourse.tile as tile
from concourse import bass_utils

F32 = mybir.dt.float32
BF16 = mybir.dt.bfloat16
AF = mybir.ActivationFunctionType
ALU = mybir.AluOpType

N_CORES = 8
K = 5
PAD = 2
B, C, H, W = 4, 3, 512, 512
N_IMGS = B * C                    # 12
HALF_ROWS = 256                   # output rows per piece
PIECE_ROWS = HALF_ROWS + 2 * PAD  # 260
PIECE_COLS = W + 2 * PAD          # 516
PIECES_PER_CORE = (N_IMGS * 2) // N_CORES  # 3

# taps grouped by uniform-stride runs of dj (whole rows; the center row
# splits around the center tap) so subs/muls batch into single DVE ops
# with overlapping-window APs
ROW_GROUPS = [
    (0, [0, 1, 2, 3, 4]),
    (1, [0, 1, 2, 3, 4]),
    (3, [0, 1, 2, 3, 4]),
    (4, [0, 1, 2, 3, 4]),
    (2, [0, 1]),
    (2, [3, 4]),
]
TAPS = [(di, dj) for di, djs in ROW_GROUPS for dj in djs]

_cached = {}


def _build(sk: np.ndarray, gamma: float) -> bass.Bass:
    """Build the per-core Bass module (SPMD: same NEFF on all 8 cores)."""
    nc = bacc.Bacc("TRN2", target_bir_lowering=False, debug=False)
    x_in = nc.dram_tensor(
        "x_in", [PIECES_PER_CORE, PIECE_ROWS, PIECE_COLS], F32, kind="ExternalInput"
    ).ap()
    ident_in = nc.dram_tensor("ident", [128, 128], F32, kind="ExternalInput").ap()
    y_out = nc.dram_tensor(
        "y_out", [PIECES_PER_CORE, HALF_ROWS, W], F32, kind="ExternalOutput"
    ).ap()

    with tile.TileContext(nc) as tc:
        with (
            tc.tile_pool(name="const_pool", bufs=1) as const_pool,
            tc.tile_pool(name="slab_pool", bufs=2) as slab_pool,
            tc.tile_pool(name="work_pool", bufs=2) as work_pool,
            tc.tile_pool(name="epi_pool", bufs=2) as epi_pool,
            tc.tile_pool(name="psum_pool", bufs=2, space="PSUM") as psum_pool,
        ):
            # sk_t-scaled identity matrices (bf16) as matmul weights
            ident_f = const_pool.tile([128, 128], F32, tag="ident_f",
                                      name="ident_f")
            nc.sync.dma_start(ident_f[:, :], ident_in)
            skI = const_pool.tile([128, len(TAPS), 128], BF16, tag="skI",
                                  name="skI")
            skI_f = const_pool.tile([128, 128], F32, tag="skI_f", name="skI_f")
            # Derivative_Erf(x) = (2/sqrt(pi)) exp(-x^2); fold the sqrt(pi)/2
            # normalization into the spatial weights
            norm = float(np.sqrt(np.pi) / 2.0)
            for tidx, (di, dj) in enumerate(TAPS):
                nc.vector.tensor_scalar(skI_f[:, :], ident_f[:, :],
                                        float(sk[di, dj]) * norm, None,
                                        op0=ALU.mult)
                nc.vector.tensor_copy(skI[:, tidx, :], skI_f[:, :])

            for p in range(PIECES_PER_CORE):
                for t in range(2):  # two 128-row tiles per 256-row piece
                    r0 = t * 128
                    # One DMA loads all 5 vertically-shifted slabs as an
                    # overlapping-window read: dest [128, 5, 516], src row
                    # (r0 + part + di).
                    slab = slab_pool.tile([128, K, PIECE_COLS], F32, tag="slab",
                                          name=f"slab_p{p}t{t}")
                    src_win = x_in[p, r0 : r0 + 128 + K - 1, :].copy()
                    src_win.ap = bass_rust.VecI64Pair(
                        [(PIECE_COLS, 128), (PIECE_COLS, K), (1, PIECE_COLS)]
                    )
                    nc.sync.dma_start(slab[:, :, :], src_win)
                    c = slab[:, PAD, PAD : PAD + W]
                    # bf16 copy of the slab for the (bf16 x bf16) wp multiply
                    # (on ACT: the DVE is the busier engine)
                    slab16 = slab_pool.tile([128, K, PIECE_COLS], BF16,
                                            tag="slab16", name=f"slab16_p{p}t{t}")
                    nc.scalar.copy(slab16[:, :, :], slab[:, :, :])

                    S_ps = psum_pool.tile([128, W], F32, tag="S",
                                          name=f"Sps_p{p}t{t}")
                    T_ps = psum_pool.tile([128, W], F32, tag="T",
                                          name=f"Tps_p{p}t{t}")

                    def win(base_tile, ncols, di, dj0, g):
                        """overlapping-window AP [128, g, 512]: tap dim strides
                        1 column."""
                        v = base_tile[:, 0, 0:W].copy()
                        v.ap = bass_rust.VecI64Pair(
                            [(K * ncols, 128), (1, g), (1, W)]
                        )
                        v.offset = base_tile[:, :, :].offset + di * ncols + dj0
                        return v

                    tidx = 0
                    for di, djs in ROW_GROUPS:
                        g = len(djs)
                        dj0 = djs[0]
                        d_ring = work_pool.tile([128, g, W], F32, tag="d",
                                                padded_shape=[128, K, W],
                                                name=f"d_p{p}t{t}r{di}_{dj0}")
                        src = win(slab, PIECE_COLS, di, dj0, g)
                        cb = c.copy()
                        cb.ap = bass_rust.VecI64Pair(
                            [(K * PIECE_COLS, 128), (0, g), (1, W)]
                        )
                        nc.vector.tensor_sub(d_ring[:, :, :], src, cb)
                        # Derivative_Erf(gamma*d) = (2/sqrt(pi)) exp(-gamma^2 d^2)
                        w_ring = work_pool.tile([128, g, W], BF16, tag="w",
                                                padded_shape=[128, K, W],
                                                name=f"w_p{p}t{t}r{di}_{dj0}")
                        nc.scalar.activation(w_ring[:, :, :], d_ring[:, :, :],
                                             AF.Derivative_Erf,
                                             scale=float(gamma))
                        wp_ring = work_pool.tile([128, g, W], BF16, tag="wp",
                                                 padded_shape=[128, K, W],
                                                 name=f"wp_p{p}t{t}r{di}_{dj0}")
                        nc.vector.tensor_mul(wp_ring[:, :, :], w_ring[:, :, :],
                                             win(slab16, PIECE_COLS, di, dj0, g))
                        for j in range(g):
                            first = tidx == 0
                            last = tidx == len(TAPS) - 1
                            nc.tensor.matmul(S_ps[:, :], skI[:, tidx, :],
                                             w_ring[:, j, :],
                                             start=first, stop=last)
                            nc.tensor.matmul(T_ps[:, :], skI[:, tidx, :],
                                             wp_ring[:, j, :],
                                             start=first, stop=last)
                            tidx += 1

                    # epilogue: fold exact center tap (w=1, wp=c) + epsilon
                    S = epi_pool.tile([128, W], F32, tag="S", name=f"S_p{p}t{t}")
                    nc.vector.tensor_scalar(S[:, :], S_ps[:, :],
                                            float(1.0 + 1e-8), None, op0=ALU.add)
                    Tc = epi_pool.tile([128, W], F32, tag="Tc", name=f"Tc_p{p}t{t}")
                    nc.vector.tensor_add(Tc[:, :], T_ps[:, :], c)
                    R = epi_pool.tile([128, W], F32, tag="R", name=f"R_p{p}t{t}")
                    scr = epi_pool.tile([128, W], F32, tag="scr",
                                        name=f"scr_p{p}t{t}")
                    nc.vector.reciprocal_approx_accurate(R[:, :], S[:, :],
                                                         scr[:, :])
                    out = epi_pool.tile([128, W], F32, tag="out",
                                        name=f"out_p{p}t{t}")
                    nc.vector.tensor_mul(out[:, :], Tc[:, :], R[:, :])
                    nc.sync.dma_start(y_out[p, r0 : r0 + 128, :], out[:, :])
    nc.compile()
    return nc


def _get_nc(sk: np.ndarray, gamma: float) -> bass.Bass:
    key = (sk.tobytes(), float(gamma))
    if _cached.get("key") != key:
        _cached["key"] = key
        _cached["nc"] = _build(sk, gamma)
    return _cached["nc"]


def kernel(x, spatial_kernel, sigma_color):
    x = np.ascontiguousarray(np.asarray(x, dtype=np.float32))
    sk = np.asarray(spatial_kernel, dtype=np.float64)
    sigma = float(np.asarray(sigma_color))

    gamma = 1.0 / (np.sqrt(2.0) * sigma)

    imgs = x.reshape(N_IMGS, H, W)
    xp = np.pad(imgs, ((0, 0), (PAD, PAD), (PAD, PAD)), mode="reflect")
    # 24 half-image pieces with halo: [24, 260, 516]
    pieces = np.stack(
        [xp[:, 0:PIECE_ROWS, :], xp[:, HALF_ROWS : HALF_ROWS + PIECE_ROWS, :]],
        axis=1,
    ).reshape(N_IMGS * 2, PIECE_ROWS, PIECE_COLS)

    nc = _get_nc(sk, gamma)
    ident = np.eye(128, dtype=np.float32)
    in_maps = [
        {
            "x_in": np.ascontiguousarray(
                pieces[PIECES_PER_CORE * k : PIECES_PER_CORE * (k + 1)]
            ),
            "ident": ident,
        }
        for k in range(N_CORES)
    ]
    trace = os.environ.get("BILATERAL_TRACE", "0") == "1"
    res = bass_utils.run_bass_kernel_spmd(
        nc, in_maps, core_ids=list(range(N_CORES)), trace=trace
    )
    kernel.last_results = res

    outs = np.stack([res.results[k]["y_out"] for k in range(N_CORES)])
    out = outs.reshape(N_IMGS, 2, HALF_ROWS, W).reshape(N_IMGS, H, W)
    return out.reshape(B, C, H, W).astype(np.float32)


kernel.last_results = None


# revision 22
# speedup vs baseline: 2.3612x; 1.0234x over previous
"""Bilateral filter (5x5, reflect pad) on 8 Trainium2 NeuronCores.

Contract: kernel(**inputs) takes the FULL inputs
  x:              [4, 3, 512, 512] f32
  spatial_kernel: [5, 5] f32
  sigma_color:    scalar f32
and returns the FULL output [4, 3, 512, 512] f32.

Sharding: pure data-parallel. The 12 images (B*C) are split into 24
half-images of 256 rows; each of the 8 cores gets 3 half-images with a
2-row halo (reflect padding applied on the host): input pieces of
[260, 516] producing output [256, 512].

Per-core kernel, per 128-row tile (24 non-center taps in 3 groups of 8):
  - one DMA loads 5 vertically-shifted slabs (overlapping-window AP)
  - per tap: d = p_t - c (DVE, f32)  into a group ring
  - per group of 8 taps (batched ACT, one instruction each):
        q = Square(gamma * d)    [128, 8*512]
        w = Exp(-q)        f32->bf16, no bias
  - per tap: wp = w * p16 (DVE bf16), then two PE matmuls with a
    sk_t-scaled identity as lhsT accumulate into PSUM:
        S_psum += sk_t * w ;  T_psum += sk_t * wp
  - center tap is exact (w=1, wp=c), folded into the epilogue:
        out = (T + c) * reciprocal(S + 1 + 1e-8)
The TensorEngine does both tap-reductions AND applies the spatial
weights (scaled-identity matmuls, f32 PSUM accumulation); batching the
activations 8-wide amortizes the ACT per-instruction overhead.
"""

import os

import numpy as np

import bass_rust
import concourse.bacc as bacc
import concourse.bass as bass
import concourse.mybir as mybir
import concourse.tile as tile
from concourse import bass_utils

F32 = mybir.dt.float32
BF16 = mybir.dt.bfloat16
AF = mybir.ActivationFunctionType
ALU = mybir.AluOpType

N_CORES = 8
K = 5
PAD = 2
B, C, H, W = 4, 3, 512, 512
N_IMGS = B * C                    # 12
HALF_ROWS = 256                   # output rows per piece
PIECE_ROWS = HALF_ROWS + 2 * PAD  # 260
PIECE_COLS = W + 2 * PAD          # 516
PIECES_PER_CORE = (N_IMGS * 2) // N_CORES  # 3

# taps grouped by uniform-stride runs of dj (whole rows; the center row
# splits around the center tap) so subs/muls batch into single DVE ops
# with overlapping-window APs
ROW_GROUPS = [
    (0, [0, 1, 2, 3, 4]),
    (1, [0, 1, 2, 3, 4]),
    (3, [0, 1, 2, 3, 4]),
    (4, [0, 1, 2, 3, 4]),
    (2, [0, 1]),
    (2, [3, 4]),
]
TAPS = [(di, dj) for di, djs in ROW_GROUPS for dj in djs]

_cached = {}


def _build(sk: np.ndarray, gamma: float) -> bass.Bass:
    """Build the per-core Bass module (SPMD: same NEFF on all 8 cores)."""
    nc = bacc.Bacc("TRN2", target_bir_lowering=False, debug=False)
    x_in = nc.dram_tensor(
        "x_in", [PIECES_PER_CORE, PIECE_ROWS, PIECE_COLS], F32, kind="ExternalInput"
    ).ap()
    ident_in = nc.dram_tensor("ident", [128, 128], F32, kind="ExternalInput").ap()
    y_out = nc.dram_tensor(
        "y_out", [PIECES_PER_CORE, HALF_ROWS, W], F32, kind="ExternalOutput"
    ).ap()

    with tile.TileContext(nc) as tc:
        with (
            tc.tile_pool(name="const_pool", bufs=1) as const_pool,
            tc.tile_pool(name="slab_pool", bufs=2) as slab_pool,
            tc.tile_pool(name="work_pool", bufs=2) as work_pool,
            tc.tile_pool(name="epi_pool", bufs=2) as epi_pool,
            tc.tile_pool(name="psum_pool", bufs=2, space="PSUM") as psum_pool,
        ):
            # sk_t-scaled identity matrices (bf16) as matmul weights
            ident_f = const_pool.tile([128, 128], F32, tag="ident_f",
                                      name="ident_f")
            nc.sync.dma_start(ident_f[:, :], ident_in)
            skI = const_pool.tile([128, len(TAPS), 128], BF16, tag="skI",
                                  name="skI")
            skI_f = const_pool.tile([128, 128], F32, tag="skI_f", name="skI_f")
            # Derivative_Erf(x) = (2/sqrt(pi)) exp(-x^2); fold the sqrt(pi)/2
            # normalization into the spatial weights
            norm = float(np.sqrt(np.pi) / 2.0)
            for tidx, (di, dj) in enumerate(TAPS):
                nc.vector.tensor_scalar(skI_f[:, :], ident_f[:, :],
                                        float(sk[di, dj]) * norm, None,
                                        op0=ALU.mult)
                nc.vector.tensor_copy(skI[:, tidx, :], skI_f[:, :])
            # bf16 identity + ones: a final I @ ones matmul adds the exact
            # center-tap weight (w=1) to S on the PE instead of a DVE op
            identb = const_pool.tile([128, 128], BF16, tag="identb",
                                     name="identb")
            nc.vector.tensor_copy(identb[:, :], ident_f[:, :])
            ones16 = const_pool.tile([128, W], BF16, tag="ones16", name="ones16")
            nc.gpsimd.memset(ones16[:, :], 1.0)

            for p in range(PIECES_PER_CORE):
                for t in range(2):  # two 128-row tiles per 256-row piece
                    r0 = t * 128
                    # One DMA loads all 5 vertically-shifted slabs as an
                    # overlapping-window read: dest [128, 5, 516], src row
                    # (r0 + part + di).
                    slab = slab_pool.tile([128, K, PIECE_COLS], F32, tag="slab",
                                          name=f"slab_p{p}t{t}")
                    src_win = x_in[p, r0 : r0 + 128 + K - 1, :].copy()
                    src_win.ap = bass_rust.VecI64Pair(
                        [(PIECE_COLS, 128), (PIECE_COLS, K), (1, PIECE_COLS)]
                    )
                    nc.sync.dma_start(slab[:, :, :], src_win)
                    c = slab[:, PAD, PAD : PAD + W]
                    # bf16 copy of the slab for the (bf16 x bf16) wp multiply
                    # (on ACT: the DVE is the busier engine)
                    slab16 = slab_pool.tile([128, K, PIECE_COLS], BF16,
                                            tag="slab16", name=f"slab16_p{p}t{t}")
                    nc.scalar.copy(slab16[:, :, :], slab[:, :, :])

                    S_ps = psum_pool.tile([128, W], F32, tag="S",
                                          name=f"Sps_p{p}t{t}")
                    T_ps = psum_pool.tile([128, W], F32, tag="T",
                                          name=f"Tps_p{p}t{t}")

                    def win(base_tile, ncols, di, dj0, g):
                        """overlapping-window AP [128, g, 512]: tap dim strides
                        1 column."""
                        v = base_tile[:, 0, 0:W].copy()
                        v.ap = bass_rust.VecI64Pair(
                            [(K * ncols, 128), (1, g), (1, W)]
                        )
                        v.offset = base_tile[:, :, :].offset + di * ncols + dj0
                        return v

                    tidx = 0
                    for di, djs in ROW_GROUPS:
                        g = len(djs)
                        dj0 = djs[0]
                        d_ring = work_pool.tile([128, g, W], F32, tag="d",
                                                padded_shape=[128, K, W],
                                                name=f"d_p{p}t{t}r{di}_{dj0}")
                        src = win(slab, PIECE_COLS, di, dj0, g)
                        cb = c.copy()
                        cb.ap = bass_rust.VecI64Pair(
                            [(K * PIECE_COLS, 128), (0, g), (1, W)]
                        )
                        nc.vector.tensor_sub(d_ring[:, :, :], src, cb)
                        # Derivative_Erf(gamma*d) = (2/sqrt(pi)) exp(-gamma^2 d^2)
                        w_ring = work_pool.tile([128, g, W], BF16, tag="w",
                                                padded_shape=[128, K, W],
                                                name=f"w_p{p}t{t}r{di}_{dj0}")
                        nc.scalar.activation(w_ring[:, :, :], d_ring[:, :, :],
                                             AF.Derivative_Erf,
                                             scale=float(gamma))
                        wp_ring = work_pool.tile([128, g, W], BF16, tag="wp",
                                                 padded_shape=[128, K, W],
                                                 name=f"wp_p{p}t{t}r{di}_{dj0}")
                        nc.vector.tensor_mul(wp_ring[:, :, :], w_ring[:, :, :],
                                             win(slab16, PIECE_COLS, di, dj0, g))
                        for j in range(g):
                            first = tidx == 0
                            last = tidx == len(TAPS) - 1
                            nc.tensor.matmul(S_ps[:, :], skI[:, tidx, :],
                                             w_ring[:, j, :],
                                             start=first, stop=False)
                            nc.tensor.matmul(T_ps[:, :], skI[:, tidx, :],
                                             wp_ring[:, j, :],
                                             start=first, stop=last)
                            tidx += 1
                    # exact center-tap weight (w=1): S += I @ ones
                    nc.tensor.matmul(S_ps[:, :], identb[:, :], ones16[:, :],
                                     start=False, stop=True)

                    # epilogue: center-tap numerator (wp = c) + division
                    Tc = epi_pool.tile([128, W], F32, tag="Tc", name=f"Tc_p{p}t{t}")
                    nc.vector.tensor_add(Tc[:, :], T_ps[:, :], c)
                    R = epi_pool.tile([128, W], F32, tag="R", name=f"R_p{p}t{t}")
                    nc.vector.reciprocal_approx_fast(R[:, :], S_ps[:, :])
                    out = epi_pool.tile([128, W], F32, tag="out",
                                        name=f"out_p{p}t{t}")
                    nc.vector.tensor_mul(out[:, :], Tc[:, :], R[:, :])
                    nc.sync.dma_start(y_out[p, r0 : r0 + 128, :], out[:, :])
    nc.compile()
    return nc


def _get_nc(sk: np.ndarray, gamma: float) -> bass.Bass:
    key = (sk.tobytes(), float(gamma))
    if _cached.get("key") != key:
        _cached["key"] = key
        _cached["nc"] = _build(sk, gamma)
    return _cached["nc"]


def kernel(x, spatial_kernel, sigma_color):
    x = np.ascontiguousarray(np.asarray(x, dtype=np.float32))
    sk = np.asarray(spatial_kernel, dtype=np.float64)
    sigma = float(np.asarray(sigma_color))

    gamma = 1.0 / (np.sqrt(2.0) * sigma)

    imgs = x.reshape(N_IMGS, H, W)
    xp = np.pad(imgs, ((0, 0), (PAD, PAD), (PAD, PAD)), mode="reflect")
    # 24 half-image pieces with halo: [24, 260, 516]
    pieces = np.stack(
        [xp[:, 0:PIECE_ROWS, :], xp[:, HALF_ROWS : HALF_ROWS + PIECE_ROWS, :]],
        axis=1,
    ).reshape(N_IMGS * 2, PIECE_ROWS, PIECE_COLS)

    nc = _get_nc(sk, gamma)
    ident = np.eye(128, dtype=np.float32)
    in_maps = [
        {
            "x_in": np.ascontiguousarray(
                pieces[PIECES_PER_CORE * k : PIECES_PER_CORE * (k + 1)]
            ),
            "ident": ident,
        }
        for k in range(N_CORES)
    ]
    trace = os.environ.get("BILATERAL_TRACE", "0") == "1"
    res = bass_utils.run_bass_kernel_spmd(
        nc, in_maps, core_ids=list(range(N_CORES)), trace=trace
    )
    kernel.last_results = res

    outs = np.stack([res.results[k]["y_out"] for k in range(N_CORES)])
    out = outs.reshape(N_IMGS, 2, HALF_ROWS, W).reshape(N_IMGS, H, W)
    return out.reshape(B, C, H, W).astype(np.float32)


kernel.last_results = None


# revision 23
# speedup vs baseline: 2.9289x; 1.2404x over previous
"""Bilateral filter (5x5, reflect pad) on 8 Trainium2 NeuronCores.

Contract: kernel(**inputs) takes the FULL inputs
  x:              [4, 3, 512, 512] f32
  spatial_kernel: [5, 5] f32
  sigma_color:    scalar f32
and returns the FULL output [4, 3, 512, 512] f32.

Sharding: pure data-parallel. The 12 images (B*C) are split into 24
half-images of 256 rows; each of the 8 cores gets 3 half-images with a
2-row halo (reflect padding applied on the host): input pieces of
[260, 516] producing output [256, 512].

Per-core kernel, per 128-row tile (24 non-center taps, fp16 datapath):
  - one DMA loads 5 vertically-shifted slabs (overlapping-window AP);
    ACT makes an fp16 copy
  - per uniform-stride tap row-group (4x5 taps + 2x2 taps), single
    batched ops via overlapping-window APs:
        d  = p - c                       (DVE fp16 TT sub, 2x mode)
        w  = Derivative_Erf(gamma * d)   (ACT: (2/sqrt(pi)) exp(-g^2 d^2))
        wp = w * p                       (DVE fp16 TT mul, 2x mode)
    into whole-tile W/WP buffers [128, 24, 512] fp16
  - one dense PE burst of 49 matmuls (sk_t-scaled fp16 identities as
    lhsT, f32 PSUM accumulation; the sqrt(pi)/2 normalization and the
    spatial weights ride in lhsT; a final I @ ones adds the exact
    center weight):
        S_psum = sum_t sk_t * w_t + 1 ;  T_psum = sum_t sk_t * wp_t
    The dense burst keeps the TensorE p-state at full clock.
  - epilogue: out = (T + c) * reciprocal_approx(S)   (center wp = c in f32)
"""

import os

import numpy as np

import bass_rust
import concourse.bacc as bacc
import concourse.bass as bass
import concourse.mybir as mybir
import concourse.tile as tile
from concourse import bass_utils

F32 = mybir.dt.float32
FP16 = mybir.dt.float16
AF = mybir.ActivationFunctionType
ALU = mybir.AluOpType

N_CORES = 8
K = 5
PAD = 2
B, C, H, W = 4, 3, 512, 512
N_IMGS = B * C                    # 12
HALF_ROWS = 256                   # output rows per piece
PIECE_ROWS = HALF_ROWS + 2 * PAD  # 260
PIECE_COLS = W + 2 * PAD          # 516
PIECES_PER_CORE = (N_IMGS * 2) // N_CORES  # 3

# taps grouped by uniform-stride runs of dj (whole rows; the center row
# splits around the center tap) so subs/muls batch into single DVE ops
# with overlapping-window APs
ROW_GROUPS = [
    (0, [0, 1, 2, 3, 4]),
    (1, [0, 1, 2, 3, 4]),
    (3, [0, 1, 2, 3, 4]),
    (4, [0, 1, 2, 3, 4]),
    (2, [0, 1]),
    (2, [3, 4]),
]
TAPS = [(di, dj) for di, djs in ROW_GROUPS for dj in djs]
NT = len(TAPS)  # 24

_cached = {}


def _build(sk: np.ndarray, gamma: float) -> bass.Bass:
    """Build the per-core Bass module (SPMD: same NEFF on all 8 cores)."""
    nc = bacc.Bacc("TRN2", target_bir_lowering=False, debug=False)
    x_in = nc.dram_tensor(
        "x_in", [PIECES_PER_CORE, PIECE_ROWS, PIECE_COLS], F32, kind="ExternalInput"
    ).ap()
    ident_in = nc.dram_tensor("ident", [128, 128], F32, kind="ExternalInput").ap()
    y_out = nc.dram_tensor(
        "y_out", [PIECES_PER_CORE, HALF_ROWS, W], F32, kind="ExternalOutput"
    ).ap()

    with tile.TileContext(nc) as tc:
        with (
            tc.tile_pool(name="const_pool", bufs=1) as const_pool,
            tc.tile_pool(name="slab_pool", bufs=2) as slab_pool,
            tc.tile_pool(name="work_pool", bufs=2) as work_pool,
            tc.tile_pool(name="epi_pool", bufs=2) as epi_pool,
            tc.tile_pool(name="psum_pool", bufs=2, space="PSUM") as psum_pool,
        ):
            # sk_t-scaled identity matrices (fp16) as matmul weights;
            # Derivative_Erf(x) = (2/sqrt(pi)) exp(-x^2), so fold sqrt(pi)/2
            # into the spatial weights
            ident_f = const_pool.tile([128, 128], F32, tag="ident_f",
                                      name="ident_f")
            nc.sync.dma_start(ident_f[:, :], ident_in)
            skI = const_pool.tile([128, NT, 128], FP16, tag="skI", name="skI")
            skI_f = const_pool.tile([128, 128], F32, tag="skI_f", name="skI_f")
            norm = float(np.sqrt(np.pi) / 2.0)
            for tidx, (di, dj) in enumerate(TAPS):
                nc.vector.tensor_scalar(skI_f[:, :], ident_f[:, :],
                                        float(sk[di, dj]) * norm, None,
                                        op0=ALU.mult)
                nc.vector.tensor_copy(skI[:, tidx, :], skI_f[:, :])
            # fp16 identity + ones: a final I @ ones matmul adds the exact
            # center-tap weight (w=1) to S on the PE instead of a DVE op
            identh = const_pool.tile([128, 128], FP16, tag="identh",
                                     name="identh")
            nc.vector.tensor_copy(identh[:, :], ident_f[:, :])
            ones16 = const_pool.tile([128, W], FP16, tag="ones16", name="ones16")
            nc.gpsimd.memset(ones16[:, :], 1.0)

            for p in range(PIECES_PER_CORE):
                for t in range(2):  # two 128-row tiles per 256-row piece
                    r0 = t * 128
                    # One DMA loads all 5 vertically-shifted slabs as an
                    # overlapping-window read: dest [128, 5, 516], src row
                    # (r0 + part + di).
                    slab = slab_pool.tile([128, K, PIECE_COLS], F32, tag="slab",
                                          name=f"slab_p{p}t{t}")
                    src_win = x_in[p, r0 : r0 + 128 + K - 1, :].copy()
                    src_win.ap = bass_rust.VecI64Pair(
                        [(PIECE_COLS, 128), (PIECE_COLS, K), (1, PIECE_COLS)]
                    )
                    nc.sync.dma_start(slab[:, :, :], src_win)
                    c = slab[:, PAD, PAD : PAD + W]
                    # fp16 copy of the slab (on ACT: DVE is the busier engine)
                    slab16 = slab_pool.tile([128, K, PIECE_COLS], FP16,
                                            tag="slab16", name=f"slab16_p{p}t{t}")
                    nc.scalar.copy(slab16[:, :, :], slab[:, :, :])

                    S_ps = psum_pool.tile([128, W], F32, tag="S",
                                          name=f"Sps_p{p}t{t}")
                    T_ps = psum_pool.tile([128, W], F32, tag="T",
                                          name=f"Tps_p{p}t{t}")
                    W_buf = work_pool.tile([128, NT, W], FP16, tag="W",
                                           name=f"W_p{p}t{t}")
                    WP_buf = work_pool.tile([128, NT, W], FP16, tag="WP",
                                            name=f"WP_p{p}t{t}")

                    def win(base_tile, di, dj0, g):
                        """overlapping-window AP [128, g, 512] on an fp16/f32
                        [128, K, 516] slab: tap dim strides 1 column."""
                        v = base_tile[:, 0, 0:W].copy()
                        v.ap = bass_rust.VecI64Pair(
                            [(K * PIECE_COLS, 128), (1, g), (1, W)]
                        )
                        v.offset = (base_tile[:, :, :].offset
                                    + di * PIECE_COLS + dj0)
                        return v

                    tidx = 0
                    for di, djs in ROW_GROUPS:
                        g = len(djs)
                        dj0 = djs[0]
                        d_ring = work_pool.tile([128, g, W], FP16, tag="d",
                                                padded_shape=[128, K, W],
                                                name=f"d_p{p}t{t}r{di}_{dj0}")
                        src = win(slab16, di, dj0, g)
                        cb = slab16[:, PAD, PAD : PAD + W].copy()
                        cb.ap = bass_rust.VecI64Pair(
                            [(K * PIECE_COLS, 128), (0, g), (1, W)]
                        )
                        nc.vector.tensor_sub(d_ring[:, :, :], src, cb)
                        # w = Derivative_Erf(gamma*d) = (2/sqrt(pi)) e^(-g^2 d^2)
                        nc.scalar.activation(W_buf[:, tidx : tidx + g, :],
                                             d_ring[:, :, :],
                                             AF.Derivative_Erf,
                                             scale=float(gamma))
                        nc.vector.tensor_mul(WP_buf[:, tidx : tidx + g, :],
                                             W_buf[:, tidx : tidx + g, :],
                                             win(slab16, di, dj0, g))
                        tidx += g

                    # dense PE burst: 49 matmuls, f32 PSUM accumulation
                    for tidx in range(NT):
                        first = tidx == 0
                        nc.tensor.matmul(S_ps[:, :], skI[:, tidx, :],
                                         W_buf[:, tidx, :],
                                         start=first, stop=False)
                        nc.tensor.matmul(T_ps[:, :], skI[:, tidx, :],
                                         WP_buf[:, tidx, :],
                                         start=first, stop=(tidx == NT - 1))
                    # exact center-tap weight (w=1): S += I @ ones
                    nc.tensor.matmul(S_ps[:, :], identh[:, :], ones16[:, :],
                                     start=False, stop=True)

                    # epilogue: center-tap numerator (wp = c, f32) + division
                    Tc = epi_pool.tile([128, W], F32, tag="Tc", name=f"Tc_p{p}t{t}")
                    nc.vector.tensor_add(Tc[:, :], T_ps[:, :], c)
                    R = epi_pool.tile([128, W], F32, tag="R", name=f"R_p{p}t{t}")
                    nc.vector.reciprocal_approx_fast(R[:, :], S_ps[:, :])
                    out = epi_pool.tile([128, W], F32, tag="out",
                                        name=f"out_p{p}t{t}")
                    nc.vector.tensor_mul(out[:, :], Tc[:, :], R[:, :])
                    nc.sync.dma_start(y_out[p, r0 : r0 + 128, :], out[:, :])
    nc.compile()
    return nc


def _get_nc(sk: np.ndarray, gamma: float) -> bass.Bass:
    key = (sk.tobytes(), float(gamma))
    if _cached.get("key") != key:
        _cached["key"] = key
        _cached["nc"] = _build(sk, gamma)
    return _cached["nc"]


def kernel(x, spatial_kernel, sigma_color):
    x = np.ascontiguousarray(np.asarray(x, dtype=np.float32))
    sk = np.asarray(spatial_kernel, dtype=np.float64)
    sigma = float(np.asarray(sigma_color))

    gamma = 1.0 / (np.sqrt(2.0) * sigma)

    imgs = x.reshape(N_IMGS, H, W)
    xp = np.pad(imgs, ((0, 0), (PAD, PAD), (PAD, PAD)), mode="reflect")
    # 24 half-image pieces with halo: [24, 260, 516]
    pieces = np.stack(
        [xp[:, 0:PIECE_ROWS, :], xp[:, HALF_ROWS : HALF_ROWS + PIECE_ROWS, :]],
        axis=1,
    ).reshape(N_IMGS * 2, PIECE_ROWS, PIECE_COLS)

    nc = _get_nc(sk, gamma)
    ident = np.eye(128, dtype=np.float32)
    in_maps = [
        {
            "x_in": np.ascontiguousarray(
                pieces[PIECES_PER_CORE * k : PIECES_PER_CORE * (k + 1)]
            ),
            "ident": ident,
        }
        for k in range(N_CORES)
    ]
    trace = os.environ.get("BILATERAL_TRACE", "0") == "1"
    res = bass_utils.run_bass_kernel_spmd(
        nc, in_maps, core_ids=list(range(N_CORES)), trace=trace
    )
    kernel.last_results = res

    outs = np.stack([res.results[k]["y_out"] for k in range(N_CORES)])
    out = outs.reshape(N_IMGS, 2, HALF_ROWS, W).reshape(N_IMGS, H, W)
    return out.reshape(B, C, H, W).astype(np.float32)


kernel.last_results = None
